# revision 36
# baseline (speedup 1.0000x reference)
"""Trainium2 Bass kernel for the BiAttention problem (v2).

Math (per batch b, L=1024, D=256):
  s0[i] = sum_d c[i,d] c_weight[d]
  s1[j] = sum_d c[j,d] q_weight[d]
  s2[i,j] = sum_d (c[i,d]*cqw[d]) q[j,d]
  S = s0 + s1 + s2 (+scalar bias: cancels in both softmaxes)
  S1 = softmax_j(S);  C2Q = S1 @ q
  S2[b,j,i] = exp(S[b,i,j]) / Z[i,j],  Z = sum_b exp(S[b])  (softmax over b)
  Q2C = S1 @ (S2 @ c)   (re-associated from (S1@S2)@c: 2x fewer flops)
  out = concat(c, C2Q, c*C2Q, c*Q2C) on axis 0.

Sharding: batch 16 over 8 cores (2 per core); the only cross-core data is
Z -> one bf16 [1024,1024] AllReduce.

Design (vs the v1 baseline, 152.5us -> 113.9us):
  * All operands bf16; the host pre-transposes (c*cqw)^T and q^T so the
    kernel has NO PE transposes and no fp32r paths, and packs the inputs
    into few DMAs (HWDGE descriptor generation serializes at ~0.6us/DMA).
  * A short dependency-free PE warmup ramps the tensor-engine p-state
    (cold/stuttering PE runs at 0.65-1.2GHz instead of 2.4GHz).
  * Phase 1: E = exp(s2 + s1[rank-1 PE matmul] + s0[ACT bias]), paced by
    the 16 serial ACT exps; Zpart staged to DRAM as it completes (batch 0
    plain HWDGE writes, batch 1 via 2 batched gpsimd accumulate-DMAs).
  * One 2MB AllReduce; its window is filled with phase2a
    (E1T = exp(s2^T + s1)) and both batches' C2Q (+ rD1 scale at evac).
  * Post-AR: rZ = 1/Z with the native DVE reciprocal directly in bf16
    (no fp32 widen/narrow), S2T = E*rZ in place, produced COLUMN-HALF-
    major (all-bf16 2x DVE muls, 6/2 of batch-1 on Pool): each finished
    column-half fully unblocks half of G3's W groups, so the GEMM rather
    than DVE production binds.  Then W = S2T^T @ c with the
    two batches' output groups interleaved so the GEMM streams against
    S2T production, and c*Q2C = ((E1T^T @ W) * rD1) * c in one fused
    DVE scalar_tensor_tensor per block (bare Q2C is never an output).
Host does only O(B*L*D) prep: GEMVs s0/s1, transposes/casts, final concat.
"""

import sys

import numpy as np
import ml_dtypes

for _p in ("/opt/trn_rl_repo",):
    if _p not in sys.path:
        sys.path.insert(0, _p)

import concourse.bacc as bacc
import concourse.bass as bass
import concourse.mybir as mybir
import concourse.tile as tile
from concourse.bass_utils import run_bass_kernel_spmd

F32 = mybir.dt.float32
BF16 = mybir.dt.bfloat16
AF = mybir.ActivationFunctionType
ALU = mybir.AluOpType

B, L, D = 16, 1024, 256
NCORES = 8
BPC = B // NCORES  # batches per core
P = 128
LB = L // P   # 8 L-blocks
DB = D // P   # 2 D-chunks

_CACHE = {}


def _build_nc():
    nc = bacc.Bacc(
        "TRN2",
        target_bir_lowering=False,
        debug=False,
        num_devices=NCORES,
    )

    # ---- kernel I/O (all bf16 except the small fp32 bias vectors) ----
    # tq2[b][t] packs [(c*cqw)^T chunk t | q^T chunk t]: 2 DMAs/batch so the
    # first phase-1 matmul only waits on the 0.5MB t=0 pair
    tq2 = nc.dram_tensor("tq2", [BPC, DB, 2, P, L], BF16, kind="ExternalInput")
    # cq2[b] packs [c | q] natural layout: [2, L, D] -> 1 DMA/batch
    cq2 = nc.dram_tensor("cq2", [BPC, 2, L, D], BF16, kind="ExternalInput")
    # sv packs s0c|s1c column layouts for both batches: [BPC, 2, P, LB] fp32
    sv_d = nc.dram_tensor("sv", [BPC, 2, P, LB], F32, kind="ExternalInput")
    s1r_d = nc.dram_tensor("s1r", [BPC, L], BF16, kind="ExternalInput")

    o_c2q = nc.dram_tensor("o_c2q", [BPC, L, D], BF16, kind="ExternalOutput")
    o_cc2q = nc.dram_tensor("o_cc2q", [BPC, L, D], BF16, kind="ExternalOutput")
    o_cq2c = nc.dram_tensor("o_cq2c", [BPC, L, D], BF16, kind="ExternalOutput")

    rg = [list(range(NCORES))]

    with tile.TileContext(nc) as tc:
        with (
            tc.tile_pool(name="dram", bufs=1, space="DRAM") as dram,
            tc.tile_pool(name="small", bufs=1) as small,
            tc.tile_pool(name="inp", bufs=1) as inp,
            tc.tile_pool(name="Ep", bufs=16) as Ep,
            tc.tile_pool(name="E1Tp", bufs=16) as E1Tp,
            tc.tile_pool(name="Zp", bufs=8) as Zpool,
            tc.tile_pool(name="Wp", bufs=16) as Wp,
            tc.tile_pool(name="st", bufs=4) as stp,
            tc.tile_pool(name="psV", bufs=2, space="PSUM") as psV,
            tc.tile_pool(name="psC", bufs=2, space="PSUM") as psC,
            tc.tile_pool(name="psQ", bufs=2, space="PSUM") as psQ,
        ):
            zin = dram.tile([L, L], BF16, name="zin")
            zout = dram.tile([L, L], BF16, name="zout", addr_space="Shared")

            # ---- bulk input loads (phase-1 operands first, batch-0 first) ----
            # TQ[b] holds [AT t0 | qT t0 | AT t1 | qT t1] as [P, 4, L]
            TQ = [inp.tile([P, DB, 2, L], BF16, name=f"TQ{b}")
                  for b in range(BPC)]
            AT = [[TQ[b][:, t, 0, :] for t in range(DB)] for b in range(BPC)]
            qT = [[TQ[b][:, t, 1, :] for t in range(DB)] for b in range(BPC)]
            for t in range(DB):
                nc.sync.dma_start(
                    TQ[0][:, t], tq2[0, t].rearrange("x p l -> p x l"))

            # PE p-state warmup: ~26 dependency-free matmuls on memset data
            # keep the tensor clock ramping to 2.4GHz while the real inputs
            # stream in (the cost model derates a cold/stuttering PE 2-4x)
            warm = small.tile([1, P], BF16, name="warm")
            nc.gpsimd.memset(warm[:], 0.0)

            def pe_warmup(n):
                for _ in range(n):
                    wps = psC.tile([P, P], F32, name="wps", tag="psc",
                                   padded_shape=[P, 512])
                    nc.tensor.matmul(wps[:], warm[0:1, :], warm[0:1, :],
                                     start=True, stop=True)

            pe_warmup(12)

            # ---- small constants / vectors (on the ACT queue, in parallel) --
            ones = small.tile([1, P], BF16, name="ones")
            nc.gpsimd.memset(ones[:], 1.0)
            s1r = small.tile([1, BPC * L], BF16, name="s1r")
            nc.scalar.dma_start(s1r[:], s1r_d.rearrange("b l -> (b l)")[None, :])
            sv = small.tile([P, BPC, 2, LB], F32, name="sv")
            nc.scalar.dma_start(sv[:], sv_d.rearrange("b x p l -> p b x l"))
            s0c = [sv[:, b, 0, :] for b in range(BPC)]
            s1c = [sv[:, b, 1, :] for b in range(BPC)]

            rsE = [small.tile([P, LB], F32, name=f"rsE{b}") for b in range(BPC)]
            es0 = [small.tile([P, LB], F32, name=f"es0{b}") for b in range(BPC)]
            rD1 = [small.tile([P, LB], F32, name=f"rD1{b}") for b in range(BPC)]
            # es0 = exp(s0) only needs the sv DMA: do it before phase 1 so the
            # rD1 chain (and everything the scheduler merges into its waits)
            # never sits behind the 16 big phase-1 exps on ACT
            for b in range(BPC):
                nc.scalar.activation(es0[b][:], s0c[b], AF.Exp)

            for t in range(DB):
                nc.sync.dma_start(
                    TQ[1][:, t], tq2[1, t].rearrange("x p l -> p x l"))
            # CQ[b] holds [c | q] natural layout as [P, 2, LB, D].
            # On the SP queue: the ACT sequencer must stay free to pace the
            # phase-1 exps (its DMACopy dispatch costs >1us each).
            CQ = [inp.tile([P, 2, LB, D], BF16, name=f"CQ{b}")
                  for b in range(BPC)]
            cnat = [CQ[b][:, 0] for b in range(BPC)]
            qnat = [CQ[b][:, 1] for b in range(BPC)]
            for b in range(BPC):
                nc.sync.dma_start(
                    CQ[b][:], cq2[b].rearrange("x (m p) d -> p x m d", p=P))

            # ---- phase 1: E = exp(s2 + s1 + s0) (batch-major so batch 1's
            # inputs stream in behind batch 0's compute), Zpart to DRAM.
            # E[0] is 8 separate tiles (per-tile deps: G3(b0) starts on the
            # first divided tile); E[1] is one supertile so the accumulating
            # Z-staging DMA can batch 4 m-blocks per transfer.
            E0 = [Ep.tile([P, L], BF16, name=f"E0_{m}", tag="E0", bufs=LB)
                  for m in range(LB)]
            Es1 = Ep.tile([P, LB, L], BF16, name="E1", tag="E1", bufs=1)
            E = [E0, [Es1[:, m, :] for m in range(LB)]]
            for b in range(BPC):
                for m in range(LB):
                    pv = psV.tile([P, L], F32, name="pv", tag="pv")
                    for n in range(2):
                        sl = slice(n * 512, (n + 1) * 512)
                        nc.tensor.matmul(
                            pv[:, sl], AT[b][0][:, m * P:(m + 1) * P],
                            qT[b][0][:, sl], start=True, stop=False,
                        )
                        nc.tensor.matmul(
                            pv[:, sl], AT[b][1][:, m * P:(m + 1) * P],
                            qT[b][1][:, sl], start=False, stop=False,
                        )
                        nc.tensor.matmul(
                            pv[:, sl], ones[0:1, :],
                            s1r[0:1, b * L + n * 512: b * L + (n + 1) * 512],
                            start=False, stop=True,
                        )
                    nc.scalar.activation(
                        E[b][m][:], pv[:], AF.Exp,
                        bias=s0c[b][:, m:m + 1],
                        accum_out=rsE[b][:, m:m + 1],
                    )
                    # stage Zpart += E: batch 0 plain per-m writes (cheap
                    # HWDGE gens), batch 1 accumulates (gpsimd SWDGE, batched
                    # x2 to amortize its ~2.3us per-DMA cost off the AR start)
                    if b == 0:
                        nc.sync.dma_start(zin[m * P:(m + 1) * P, :],
                                          E[0][m][:])
                    elif m in (1, 3, 5):
                        nc.gpsimd.dma_start(
                            zin[(m - 1) * P:(m + 1) * P, :].rearrange(
                                "(k p) j -> p k j", p=P),
                            Es1[:, m - 1:m + 1, :],
                            accum_op=ALU.add,
                        )
                    elif m >= 6:
                        # last blocks go singly: the final (AR-gating)
                        # transfer is half the size
                        nc.gpsimd.dma_start(
                            zin[m * P:(m + 1) * P, :], E[1][m][:],
                            accum_op=ALU.add,
                        )

            # ---- cross-batch softmax denominator AllReduce ----
            nc.gpsimd.collective_compute(
                "AllReduce", ALU.add, replica_groups=rg,
                ins=[zin.opt()], outs=[zout.opt()],
            )

            # Everything below is scheduler-staged AFTER the phase-1/AR
            # critical path so the list scheduler cannot hoist it (and its
            # coarsened semaphore waits) in front of the zin staging.
            stage2 = tc.tile_wait_until(1)
            stage2.__enter__()

            # per-batch softmax scale 1/D1 = exp(s0)/rowsum(E)
            for b in range(BPC):
                nc.vector.reciprocal_approx_fast(out=rsE[b][:], in_=rsE[b][:])
                nc.vector.tensor_mul(rD1[b][:], rsE[b][:], es0[b][:])

            # ---- AR window: E1T = exp(s2^T + s1), then C2Q(b0) ----
            E1T = [[None] * LB for _ in range(BPC)]
            for b in range(BPC):
                for jm in range(LB):
                    pv = psV.tile([P, L], F32, name="pvt", tag="pv")
                    for n in range(2):
                        sl = slice(n * 512, (n + 1) * 512)
                        nc.tensor.matmul(
                            pv[:, sl], qT[b][0][:, jm * P:(jm + 1) * P],
                            AT[b][0][:, sl], start=True, stop=False,
                        )
                        nc.tensor.matmul(
                            pv[:, sl], qT[b][1][:, jm * P:(jm + 1) * P],
                            AT[b][1][:, sl], start=False, stop=True,
                        )
                    E1T[b][jm] = E1Tp.tile([P, L], BF16, name=f"E1T{b}_{jm}",
                                           tag="E1T")
                    nc.scalar.activation(
                        E1T[b][jm][:], pv[:], AF.Exp, bias=s1c[b][:, jm:jm + 1]
                    )

            # staged output supertiles: one DMA per (tensor, batch)
            c2qg = [stp.tile([P, LB, D], BF16, name=f"c2qg{b}", tag="c2qg",
                             bufs=1) for b in range(BPC)]
            cxg = [stp.tile([P, LB, D], BF16, name=f"cxg{b}", tag="cxg",
                            bufs=1) for b in range(BPC)]

            def c2q_block(b):
                for m in range(LB):
                    ps = psC.tile([P, D], F32, name="psc", tag="psc",
                                  padded_shape=[P, 512])
                    for jk in range(LB):
                        nc.tensor.matmul(
                            ps[:], E1T[b][jk][:, m * P:(m + 1) * P],
                            qnat[b][:, jk, :],
                            start=(jk == 0), stop=(jk == LB - 1),
                        )
                    nc.vector.tensor_scalar(
                        out=c2qg[b][:, m, :], in0=ps[:],
                        scalar1=rD1[b][:, m:m + 1],
                        scalar2=None, op0=ALU.mult,
                    )
                    nc.vector.tensor_mul(cxg[b][:, m, :], c2qg[b][:, m, :],
                                         cnat[b][:, m, :])
                nc.sync.dma_start(
                    o_c2q[b].rearrange("(m p) d -> p m d", p=P), c2qg[b][:])
                nc.sync.dma_start(
                    o_cc2q[b].rearrange("(m p) d -> p m d", p=P), cxg[b][:])

            c2q_block(0)
            c2q_block(1)

            # keepalive: the AR tail leaves PE with no eligible work; idle
            # resets the p-state ramp and G3 would restart at 0.65-1.2GHz

            stage2.__exit__(None, None, None)
            stage3 = tc.tile_wait_until(2)
            stage3.__enter__()

            # ---- post-AR: rZ = 1/Z via the native DVE reciprocal directly
            # in bf16 (no fp32 widen/narrow legs), then S2T = E * rZ in
            # place with all-bf16 muls (DVE 2x mode).  b0 on DVE right after
            # each reciprocal so G3(b0) streams; b1 split DVE/Pool. ----
            # Column-half-major: all tiles' first 512 columns, then the
            # second halves.  G3's stationary slices are 128-col, so a
            # finished column-half fully unblocks half of G3's W groups --
            # PE becomes the binding resource, not DVE production.
            Zts = []
            for m in range(LB):
                zb = stp.tile([P, L], BF16, name="zb", tag="zb", bufs=3)
                nc.sync.dma_start(zb[:], zout[m * P:(m + 1) * P, :])
                Zts.append(zb)
            zr = [Zpool.tile([P, L], BF16, name=f"z{m}", tag="z")
                  for m in range(LB)]
            for h in range(2):
                sl = slice(h * 512, (h + 1) * 512)
                for m in range(LB):
                    with nc.allow_low_precision("bf16 1/Z: feeds bf16 GEMMs"):
                        nc.vector.reciprocal(zr[m][:, sl], Zts[m][:, sl])
                    nc.vector.tensor_mul(E[0][m][:, sl], E[0][m][:, sl],
                                         zr[m][:, sl])
                    if m < 6:
                        nc.gpsimd.tensor_mul(E[1][m][:, sl], E[1][m][:, sl],
                                             zr[m][:, sl])
                    else:
                        nc.vector.tensor_mul(E[1][m][:, sl], E[1][m][:, sl],
                                             zr[m][:, sl])

            # ---- W = S2T^T @ c ; Q2C = (E1T^T @ W) * rD1.  The two
            # batches' W groups interleave so G3 streams against the S2T
            # production for both batches at once. ----
            Wb = [[], []]
            for jm in range(LB):
                for b in range(BPC):
                    ps = psC.tile([P, D], F32, name="psw", tag="psc",
                                  padded_shape=[P, 512])
                    for ik in range(LB):
                        nc.tensor.matmul(
                            ps[:], E[b][ik][:, jm * P:(jm + 1) * P],
                            cnat[b][:, ik, :],
                            start=(ik == 0), stop=(ik == LB - 1),
                        )
                    wt = Wp.tile([P, D], BF16, name=f"W{b}_{jm}", tag="W")
                    nc.scalar.copy(wt[:], ps[:])
                    Wb[b].append(wt)
            for b in range(BPC):
                W = Wb[b]
                for m in range(LB):
                    ps = psQ.tile([P, D], F32, name="psq", tag="psq",
                                  padded_shape=[P, 512])
                    for jk in range(LB):
                        nc.tensor.matmul(
                            ps[:], E1T[b][jk][:, m * P:(m + 1) * P], W[jk][:],
                            start=(jk == 0), stop=(jk == LB - 1),
                        )
                    # c*Q2C directly: (psum * rD1) * c fused in one DVE op
                    # (bare Q2C is never an output, so no intermediate)
                    cx2t = stp.tile([P, D], BF16, name="cx2t", tag="cx2")
                    nc.vector.scalar_tensor_tensor(
                        out=cx2t[:], in0=ps[:], scalar=rD1[b][:, m:m + 1],
                        in1=cnat[b][:, m, :], op0=ALU.mult, op1=ALU.mult,
                    )
                    nc.sync.dma_start(o_cq2c[b, m * P:(m + 1) * P, :],
                                      cx2t[:])

            stage3.__exit__(None, None, None)

    nc.compile()
    return nc


def _get_nc():
    if "nc" not in _CACHE:
        _CACHE["nc"] = _build_nc()
    return _CACHE["nc"]


def kernel(c, q, c_mask=None, q_mask=None, c_weight=None, q_weight=None,
           cq_weight=None, bias=None, _trace=False, **_ignored):
    BF = ml_dtypes.bfloat16
    c = np.ascontiguousarray(np.asarray(c, dtype=np.float32))
    q = np.ascontiguousarray(np.asarray(q, dtype=np.float32))
    c_weight = np.asarray(c_weight, dtype=np.float32).reshape(D, 1)
    q_weight = np.asarray(q_weight, dtype=np.float32).reshape(D, 1)
    cq_weight = np.asarray(cq_weight, dtype=np.float32).reshape(D)

    # Host-side tiny GEMVs + layout prep (the device does the ~34 GFLOP part).
    s0 = (c @ c_weight)[:, :, 0]  # [B, L]
    s1 = (c @ q_weight)[:, :, 0]  # [B, L]
    # column layout [128, LB]: partition p of block m holds index m*128+p
    sv = np.empty((B, 2, P, LB), dtype=np.float32)
    sv[:, 0] = s0.reshape(B, LB, P).transpose(0, 2, 1)
    sv[:, 1] = s1.reshape(B, LB, P).transpose(0, 2, 1)
    # tq[b][t]: [AT chunk t | qT chunk t], AT = (c*cqw)^T, each [128, L]
    tq = np.empty((B, DB, 2, P, L), dtype=BF)
    tq[:, :, 0] = (c * cq_weight).transpose(0, 2, 1).reshape(
        B, DB, P, L).astype(BF)
    tq[:, :, 1] = q.transpose(0, 2, 1).reshape(B, DB, P, L).astype(BF)
    # cq[b]: [c | q] natural
    cq = np.empty((B, 2, L, D), dtype=BF)
    cq[:, 0] = c.astype(BF)
    cq[:, 1] = q.astype(BF)
    s1rb = s1.astype(BF)

    nc = _get_nc()
    in_maps = []
    for k in range(NCORES):
        sl = slice(k * BPC, (k + 1) * BPC)
        in_maps.append({
            "tq2": tq[sl],
            "cq2": cq[sl],
            "sv": np.ascontiguousarray(sv[sl]),
            "s1r": s1rb[sl],
        })

    res = run_bass_kernel_spmd(
        nc, in_maps, core_ids=list(range(NCORES)), trace=_trace
    )
    _CACHE["last_result"] = res

    out = np.empty((4 * B, L, D), dtype=np.float32)
    out[0:B] = c
    for k in range(NCORES):
        sl = slice(k * BPC, (k + 1) * BPC)
        r = res.results[k]
        out[B:2 * B][sl] = np.asarray(r["o_c2q"]).astype(np.float32)
        out[2 * B:3 * B][sl] = np.asarray(r["o_cc2q"]).astype(np.float32)
        out[3 * B:4 * B][sl] = np.asarray(r["o_cq2c"]).astype(np.float32)
    return out

